# revision 18
# baseline (speedup 1.0000x reference)
"""Trainium2 Bass kernel: gamma-scaled negative squared-distance matrix.

Computes out[b,k] = -gamma[k] * (||D[b]||^2 + ||W[k]||^2 - 2*D[b].W[k])
for D [16384,512], W [1000,512], gamma [1000] -> out [16384,1000] fp32.

Strategy
--------
Data-parallel over 8 NeuronCores: D sharded along batch (2048 rows/core),
weights/gamma replicated. Everything is folded into one PE accumulation:

  psum[b,k] = sum_f DT[f,b]*WT2[f,k]      (bf16, WT2 = (2*gamma*W)^T)
            + 1 * (-gamma[k]*w_sq[k])     (fp32 aug row 0)
            + d_sq[b] * (-gamma[k])       (fp32 aug row 1)
            = out[b,k]

so the only non-matmul device work is the PSUM->SBUF copy before DMA out.
Host-side prep (cheap, O(B*F)): transpose D, fold gamma into W, compute
row norms in fp64. bf16 operand error is ~1e-4 relative (fp32 aug rows
keep the cancellation-sensitive norm terms exact).
"""

import os
import sys
import types

sys.path.insert(0, "/opt/trn_rl_repo")

import numpy as np
import ml_dtypes


def _install_ntff_hook():
    """The agent image's ``antenv`` lacks ``axon_hooks``; synthesize it and
    register the ctypes NTFF profile hook so trace=True works (and so a
    BASS_TRACE=1 environment doesn't crash the import in bass_utils)."""
    try:
        import antenv.axon_hooks  # noqa: F401

        return
    except ImportError:
        pass
    try:
        import antenv

        mod = types.ModuleType("antenv.axon_hooks")
        mod._hook = None
        mod.set_axon_ntff_profile_hook = lambda h: setattr(mod, "_hook", h)
        mod.get_axon_ntff_profile_hook = lambda: mod._hook
        sys.modules["antenv.axon_hooks"] = mod
        antenv.axon_hooks = mod
        so = "/opt/axon/libaxon_pjrt.so"
        if os.path.exists(so):
            from trn_agent_boot.trn_boot import _ntff_profile_via_ctypes

            mod._hook = _ntff_profile_via_ctypes(so)
    except Exception:
        pass


_install_ntff_hook()

import concourse.bass as bass
import concourse.tile as tile
from concourse import bacc, mybir
from concourse import bass_utils

B, F, K = 16384, 512, 1000
NCORES = 8
BS = B // NCORES          # 2048 batch rows per core
P = 128                   # partitions
FC = F // P               # 4 contraction chunks
BT = BS // P              # 16 batch tiles per core
KT0, KT1 = 500, 500       # two k-tiles (<=512 fp32 psum bank)
K_TILES = ((0, KT0), (KT0, KT1))

_NC_CACHE = None


def _build_nc():
    nc = bacc.Bacc("TRN2", target_bir_lowering=False, debug=False)

    bf16 = mybir.dt.bfloat16
    f32 = mybir.dt.float32

    dt = nc.dram_tensor("dt", [F, BS], bf16, kind="ExternalInput").ap()
    wt = nc.dram_tensor("wt", [F, K], bf16, kind="ExternalInput").ap()
    am = nc.dram_tensor("am", [4, BS], bf16, kind="ExternalInput").ap()
    an = nc.dram_tensor("an", [4, K], bf16, kind="ExternalInput").ap()
    o = nc.dram_tensor("o", [BS, K], f32, kind="ExternalOutput").ap()

    # DRAM views putting the f-chunk index into the free dim so a single
    # DMA instruction can fill one [128, FC*X] SBUF tile (DMA issue costs
    # ~700ns on the sync queue — fewer, bigger transfers win).
    dt_v = dt.rearrange("(c p) b -> p c b", p=P)   # [128, FC, BS]
    wt_v = wt.rearrange("(c p) k -> p c k", p=P)   # [128, FC, K]
    o_v = o.rearrange("(t p) k -> p t k", p=P)     # [128, BT, K]

    with tile.TileContext(nc) as tc:
        with (
            tc.tile_pool(name="ins", bufs=1) as ins_pool,
            tc.tile_pool(name="outs", bufs=3) as out_pool,
            tc.tile_pool(name="psum", bufs=4, space="PSUM") as psum_pool,
        ):
            wt_sb = ins_pool.tile([P, FC, K], bf16, name="wt_sb", tag="wt_sb")
            nc.sync.dma_start(wt_sb[:], wt_v)
            am_sb = ins_pool.tile([4, BS], bf16, name="am_sb", tag="am_sb")
            nc.sync.dma_start(am_sb[:], am[:])
            an_sb = ins_pool.tile([4, K], bf16, name="an_sb", tag="an_sb")
            nc.sync.dma_start(an_sb[:], an[:])
            # D loads ride the gpsimd (SWDGE) queue in batch-progressive
            # quarters, overlapping the weight loads on the sync queue.
            dt_sb = ins_pool.tile([P, FC, BS], bf16, name="dt_sb", tag="dt_sb")
            QB = BS // 4
            for q in range(4):
                qsl = slice(q * QB, (q + 1) * QB)
                nc.sync.dma_start(dt_sb[:, :, qsl], dt_v[:, :, qsl])

            # PE warm-up: HAM un-throttles (1.2 -> 2.4 GHz) only after ~3.4us
            # of sustained matmul activity. Burn dummy matmuls on scratch
            # SBUF while the input DMAs are in flight so the real matmuls
            # start at full clock.
            warm_in = ins_pool.tile([P, 512], bf16, name="warm_in", tag="warm_in")
            nc.gpsimd.memset(warm_in[:], 0.0)
            warm_ps = psum_pool.tile([P, 512], f32, name="warm_ps", tag="warm_ps", bufs=1)
            NWARM = 12
            for w in range(NWARM):
                nc.tensor.matmul(
                    warm_ps[:, :448],
                    warm_in[:, :P],
                    warm_in[:, :448],
                    start=(w == 0),
                    stop=(w == NWARM - 1),
                )

            for pi in range(BT // 2):
                ot = out_pool.tile([P, 2, K], f32, name="ot", tag="ot")
                for sub in range(2):
                    bi = 2 * pi + sub
                    bsl = slice(bi * P, (bi + 1) * P)
                    ps0 = psum_pool.tile([P, K_TILES[0][1]], f32, name="ps0", tag="ps0", bufs=3)
                    ps1 = psum_pool.tile([P, K_TILES[1][1]], f32, name="ps1", tag="ps1", bufs=3)
                    pss = (ps0, ps1)
                    for c in range(FC):
                        for ki, (k0, kn) in enumerate(K_TILES):
                            nc.tensor.matmul(
                                pss[ki][:],
                                dt_sb[:, c, bsl],
                                wt_sb[:, c, k0 : k0 + kn],
                                start=(c == 0),
                                stop=False,
                            )
                    for ki, (k0, kn) in enumerate(K_TILES):
                        nc.tensor.matmul(
                            pss[ki][:],
                            am_sb[:, bsl],
                            an_sb[:, k0 : k0 + kn],
                            start=False,
                            stop=True,
                        )
                        nc.vector.tensor_copy(ot[:, sub, k0 : k0 + kn], pss[ki][:])
                nc.sync.dma_start(o_v[:, 2 * pi : 2 * pi + 2, :], ot[:])

    nc.compile()
    return nc


def _get_nc():
    global _NC_CACHE
    if _NC_CACHE is None:
        _NC_CACHE = _build_nc()
    return _NC_CACHE


def _prep_in_maps(D, weight, gamma):
    D = np.asarray(D, dtype=np.float32)
    weight = np.asarray(weight, dtype=np.float32)
    gamma = np.asarray(gamma, dtype=np.float32)

    bf16 = ml_dtypes.bfloat16
    DT = np.ascontiguousarray(D.T).astype(bf16)                  # [F, B]
    WT2 = np.ascontiguousarray((2.0 * gamma[:, None] * weight).T).astype(bf16)
    d_sq = np.square(D, dtype=np.float64).sum(axis=1).astype(np.float32)
    w_sq = np.square(weight, dtype=np.float64).sum(axis=1)

    # Compensated bf16 augmentation: target  -gamma*(w_sq + d_sq)  as
    #   [1,1,r_hi,r_lo]^T . [c_hi,c_lo,-gb,-gb]  with d_sq = 512 + r,
    #   c = -gamma*(w_sq+512).  hi/lo bf16 splits keep ~1e-4 accuracy.
    gb = gamma.astype(bf16).astype(np.float32)
    c = (-gamma.astype(np.float64) * (w_sq + 512.0)).astype(np.float32)
    c_hi = c.astype(bf16).astype(np.float32)
    c_lo = c - c_hi
    r = d_sq - 512.0
    r_hi = r.astype(bf16).astype(np.float32)
    r_lo = r - r_hi

    AM = np.stack(
        [np.ones(B, np.float32), np.ones(B, np.float32), r_hi, r_lo]
    ).astype(bf16)
    AN = np.stack([c_hi, c_lo, -gb, -gb]).astype(bf16)

    in_maps = []
    for c in range(NCORES):
        sl = slice(c * BS, (c + 1) * BS)
        in_maps.append(
            {
                "dt": np.ascontiguousarray(DT[:, sl]),
                "wt": WT2,
                "am": np.ascontiguousarray(AM[:, sl]),
                "an": AN,
            }
        )
    return in_maps


def kernel_with_results(D, weight, gamma, trace=False):
    """Run on 8 cores; returns (full_output, BassKernelResults)."""
    nc = _get_nc()
    in_maps = _prep_in_maps(D, weight, gamma)
    res = bass_utils.run_bass_kernel_spmd(
        nc, in_maps, core_ids=list(range(NCORES)), trace=trace
    )
    out = np.concatenate([r["o"] for r in res.results], axis=0)
    return out, res


def kernel(D, weight, gamma):
    out, _ = kernel_with_results(D, weight, gamma)
    return out
